# revision 9
# baseline (speedup 1.0000x reference)
"""Unfold/im2col kernel for Trainium2 (Bass/Tile), 8-core data parallel.

Problem: x [4, 64, 224, 224] f32 -> out [4, 576, 49729] f32 where
out[b, (c*3+kh)*3+kw, oh*223+ow] = pad(x,1)[b, c, oh+kh, ow+kw]
(3x3 kernel, pad 1, stride 1, dilation 1, oh=ow=223).

Sharding: 8 cores = (batch 4) x (channel half 2). Each core handles
32 channels -> [288, 49729] independently; outputs concatenate on the
channel axis (channel-major row layout makes halves contiguous).

Layout strategy: the old kernel stored straight from a row-partitioned
image, capping every store descriptor at one 223-element window row
(892 B); SDMA per-descriptor overhead then limited aggregate BW
(308 us measured). Here the padded input is loaded with partition =
(row-block, channel): each of the 128 partitions holds 58 padded rows
x 226 of one channel (4 blocks of 56 output rows cover oh=223 with a
2-row halo). The scalar (Activation) and vector (DVE) engines then
materialize each of the 9 (kh, kw) shifted windows into staging tiles
where output rows are contiguous per partition; stores move CROW=14
output rows (12.5 KB) per descriptor. >=26 KB descriptors crash the
device (NRT level, found empirically), 12.5/26 KB are safe.

Descriptor sizing: loads are split into separate chunk DMAs; staging
chunks are separated by a 1-element gap so the DMA AP optimizer cannot
re-merge them into bigger descriptors (the symbolic-AP lowering path
ignores max_dma_last_dim).

Pipeline-fill tuning (measured 284 us with the naive schedule: loads
~35 us at 207 GB/s, an 8 us dead gap waiting on the first window's
copy, then stores ramping 210->270 GB/s as queue backlog builds):
loads are issued low-rows-first so the first windows' copies (which
only read rows 0..29) start while row-30+ loads are still in flight;
each window's copy chunks alternate between both engines; windows 0-1
store at (block, chunk-pair) granularity so the store stream starts
~20 us in, overlapping the tail of the loads; 3 staging buffers keep
copies ahead of the store queue.
"""

from contextlib import ExitStack

import numpy as np

import concourse.bass as bass
import concourse.tile as tile
from concourse import mybir
from concourse.ap import AP
from concourse.bass_utils import run_bass_kernel_spmd

B, C, IH, IW = 4, 64, 224, 224
N_CORES = 8
CPC = C // 2          # channels per core: 32
PH = IH + 2           # padded height/width: 226
OH = IH - 1           # output spatial: 223
OSZ = OH * OH         # 49729
NROW = CPC * 9        # 288 output rows per core
PIMG = PH * PH        # padded image elements: 51076
NBLK = 4              # output-row blocks (partition = blk*32 + c)
BSTEP = 56            # output rows per block (last block: 55)
BR = BSTEP + 2        # padded rows loaded per partition: 58
FIMG = BR * PH        # input free elems per partition: 13108
NB = 3                # staging buffers

LROWS = 29            # padded rows per load chunk DMA (29*226*4 = 26216 B)
CROW = 14             # staged rows per store chunk     (14*223*4 = 12488 B)

NCH = -(-BSTEP // CROW)        # chunks per window per partition: 4
CHST = CROW * OH + 1           # gapped chunk stride (elems)
SFREE = NCH * CHST             # staging free elems per partition

_NC_CACHE = {}


def build_nc() -> bass.Bass:
    nc = bass.Bass(dynamic_dma_scratch_size=4096)
    x = nc.declare_dram_parameter("xp", [CPC, PH, PH], mybir.dt.float32, isOutput=False)
    out = nc.declare_dram_parameter("out", [NROW, OSZ], mybir.dt.float32, isOutput=True)
    xb = x[:, :, :]
    ob = out[:, :]

    with tile.TileContext(nc) as tc:
        with ExitStack() as ctx:
            pool = ctx.enter_context(tc.tile_pool(name="p", bufs=1))
            timg = pool.tile([NBLK * CPC, FIMG], mybir.dt.float32,
                             name="timg", tag="timg")[:, :]
            stg = [pool.tile([NBLK * CPC, SFREE], mybir.dt.float32,
                             name=f"stg{i}", tag=f"stg{i}")[:, :] for i in range(NB)]

            # Loads: partition blk*32+c <- xp[c, blk*56 : blk*56+58, :], in
            # LROWS-row chunk DMAs, low rows of every block first so early
            # window copies (rows 0..29) unblock before the tail rows land.
            for r in range(0, BR, LROWS):
                n = min(LROWS, BR - r)
                for blk in range(NBLK):
                    src = AP(xb.tensor, xb.offset + (blk * BSTEP + r) * PH,
                             [[PIMG, CPC], [1, n * PH]])
                    dst = AP(timg.tensor,
                             timg.offset + blk * CPC * FIMG + r * PH,
                             [[FIMG, CPC], [1, n * PH]])
                    nc.gpsimd.dma_start(out=dst, in_=src)

            def emit_store(s, kh, kw, blk, ch0, nrows):
                """Store nrows output rows starting at chunk ch0 of block blk."""
                nfull, rem = divmod(nrows, CROW)
                base_s = s.offset + blk * CPC * SFREE + ch0 * CHST
                base_d = (ob.offset + (kh * 3 + kw) * OSZ
                          + (blk * BSTEP + ch0 * CROW) * OH)
                if nfull:
                    ssrc = AP(s.tensor, base_s,
                              [[SFREE, CPC], [CHST, nfull], [1, CROW * OH]])
                    sdst = AP(ob.tensor, base_d,
                              [[9 * OSZ, CPC], [CROW * OH, nfull], [1, CROW * OH]])
                    nc.gpsimd.dma_start(out=sdst, in_=ssrc)
                if rem:
                    ssrc = AP(s.tensor, base_s + nfull * CHST,
                              [[SFREE, CPC], [1, rem * OH]])
                    sdst = AP(ob.tensor, base_d + nfull * CROW * OH,
                              [[9 * OSZ, CPC], [1, rem * OH]])
                    nc.gpsimd.dma_start(out=sdst, in_=ssrc)

            # For each (kh, kw): engine-copy the shifted window into staging
            # (row r of the block -> padded row kh+r, cols kw..kw+222; same
            # local offsets in every partition), one copy per CROW-row chunk
            # alternating engines, then store per block (first two windows:
            # per block half, so the store stream starts during the loads).
            for w in range(9):
                kh, kw = divmod(w, 3)
                s = stg[w % NB]
                for ch in range(NCH):
                    csrc = AP(timg.tensor,
                              timg.offset + (kh + ch * CROW) * PH + kw,
                              [[FIMG, NBLK * CPC], [PH, CROW], [1, OH]])
                    cdst = AP(s.tensor, s.offset + ch * CHST,
                              [[SFREE, NBLK * CPC], [OH, CROW], [1, OH]])
                    if (w + ch) % 2 == 0:
                        nc.scalar.copy(out=cdst, in_=csrc)
                    else:
                        nc.vector.tensor_copy(cdst, csrc)
                for blk in range(NBLK):
                    nv = min(BSTEP, OH - blk * BSTEP)   # 56,56,56,55
                    if w < 2:
                        half = 2 * CROW
                        emit_store(s, kh, kw, blk, 0, half)
                        emit_store(s, kh, kw, blk, 2, nv - half)
                    else:
                        emit_store(s, kh, kw, blk, 0, nv)
    return nc


def _split_multi_waits(nc: bass.Bass) -> None:
    """Walrus allows only one sync-wait command per instruction (the
    kernel-tail drain ends up with one per DMA-completion sem lane).
    Hoist all but the last wait onto fresh single-wait NOPs inserted
    just before the instruction on the same engine — semantically
    identical (the engine blocks on each wait in turn)."""
    from bass_rust import SyncInfo

    k = 0
    for fn in nc.m.functions:
        for blk in fn.blocks:
            insts = blk.instructions
            for idx in range(len(insts) - 1, -1, -1):
                inst = insts[idx]
                si = inst.sync_info
                if si is None or len(si.on_wait) <= 1:
                    continue
                waits = list(si.on_wait)
                for w in waits[:-1]:
                    nop = mybir.InstNoOp(name=f"WSPLIT-{k}")
                    k += 1
                    nop.engine = inst.engine
                    nop.sync_info = SyncInfo(on_wait=[w], on_update=[])
                    insts.insert(idx, nop)
                si.on_wait = [waits[-1]]
                inst.sync_info = si


def get_nc() -> bass.Bass:
    if "nc" not in _NC_CACHE:
        nc = build_nc()
        _split_multi_waits(nc)
        _NC_CACHE["nc"] = nc
    return _NC_CACHE["nc"]


def make_in_maps(x: np.ndarray) -> list[dict]:
    x = np.asarray(x, dtype=np.float32)
    xp = np.pad(x, ((0, 0), (0, 0), (1, 1), (1, 1)))
    maps = []
    for core in range(N_CORES):
        b, half = divmod(core, 2)
        maps.append({"xp": np.ascontiguousarray(xp[b, half * CPC:(half + 1) * CPC])})
    return maps


def gather_out(results: list[dict]) -> np.ndarray:
    out = np.empty((B, C * 9, OSZ), dtype=np.float32)
    for core in range(N_CORES):
        b, half = divmod(core, 2)
        out[b, half * NROW:(half + 1) * NROW] = results[core]["out"]
    return out


def kernel(**inputs) -> np.ndarray:
    x = inputs["x"]
    nc = get_nc()
    res = run_bass_kernel_spmd(nc, make_in_maps(x), list(range(N_CORES)))
    return gather_out(res.results)


# revision 10
# speedup vs baseline: 1.1668x; 1.1668x over previous
"""Unfold/im2col kernel for Trainium2 (Bass/Tile), 8-core data parallel.

Problem: x [4, 64, 224, 224] f32 -> out [4, 576, 49729] f32 where
out[b, (c*3+kh)*3+kw, oh*223+ow] = pad(x,1)[b, c, oh+kh, ow+kw]
(3x3 kernel, pad 1, stride 1, dilation 1, oh=ow=223).

Sharding: 8 cores = (batch 4) x (channel half 2). Each core handles
32 channels -> [288, 49729] independently; outputs concatenate on the
channel axis (channel-major row layout makes halves contiguous).

Layout strategy: the old kernel stored straight from a row-partitioned
image, capping every store descriptor at one 223-element window row
(892 B); SDMA per-descriptor overhead then limited aggregate BW
(308 us measured). Here the padded input is loaded with partition =
(row-block, channel): each of the 128 partitions holds 58 padded rows
x 226 of one channel (4 blocks of 56 output rows cover oh=223 with a
2-row halo). The scalar (Activation) and vector (DVE) engines then
materialize each of the 9 (kh, kw) shifted windows into staging tiles
where output rows are contiguous per partition; stores move CROW=28
output rows (25 KB) per descriptor. ~50 KB descriptors crash the
device (NRT level, found empirically); 12.5-27 KB are safe.

Descriptor sizing: loads are split into separate chunk DMAs; staging
chunks are separated by a 1-element gap so the DMA AP optimizer cannot
re-merge them into bigger descriptors (the symbolic-AP lowering path
ignores max_dma_last_dim).

Schedule (from trace analysis: SDMA engines are ~99% busy start to
finish, but ns/KB falls from ~70 early to ~52 late; stores previously
could not start until ~40 us because all 8 load DMAs' descriptors
interleave across the 16 engines and so all complete together):
  - load group0 (padded rows 0-28) first; group1 (rows 28-57)
    deliberately rewrites row 28 so the WAW overlap forces Tile to
    serialize it after group0 -> group0 lands ~17 us.
  - the three kh=0 windows' low chunks copy right then, and their
    stores start ~20 us, overlapping group1's transfers.
  - group1 load DMAs are interleaved into the early store stream to
    spread prep; high chunks copy as group1 lands.
  - windows 3-8 proceed copy->store with 3 staging buffers; chunk
    copies alternate between the Activation and DVE engines.
"""

from contextlib import ExitStack

import numpy as np

import concourse.bass as bass
import concourse.tile as tile
from concourse import mybir
from concourse.ap import AP
from concourse.bass_utils import run_bass_kernel_spmd

B, C, IH, IW = 4, 64, 224, 224
N_CORES = 8
CPC = C // 2          # channels per core: 32
PH = IH + 2           # padded height/width: 226
OH = IH - 1           # output spatial: 223
OSZ = OH * OH         # 49729
NROW = CPC * 9        # 288 output rows per core
PIMG = PH * PH        # padded image elements: 51076
NBLK = 4              # output-row blocks (partition = blk*32 + c)
BSTEP = 56            # output rows per block (last block: 55)
BR = BSTEP + 2        # padded rows loaded per partition: 58
FIMG = BR * PH        # input free elems per partition: 13108
NB = 3                # staging buffers

CROW = 28             # staged rows per store chunk (28*223*4 = 24976 B)

NCH = -(-BSTEP // CROW)        # chunks per window per partition: 2
CHST = CROW * OH + 1           # gapped chunk stride (elems)
SFREE = NCH * CHST             # staging free elems per partition

_NC_CACHE = {}


def build_nc() -> bass.Bass:
    nc = bass.Bass(dynamic_dma_scratch_size=4096)
    x = nc.declare_dram_parameter("xp", [CPC, PH, PH], mybir.dt.float32, isOutput=False)
    out = nc.declare_dram_parameter("out", [NROW, OSZ], mybir.dt.float32, isOutput=True)
    xb = x[:, :, :]
    ob = out[:, :]

    with tile.TileContext(nc) as tc:
        with ExitStack() as ctx:
            pool = ctx.enter_context(tc.tile_pool(name="p", bufs=1))
            timg = pool.tile([NBLK * CPC, FIMG], mybir.dt.float32,
                             name="timg", tag="timg")[:, :]
            stg = [pool.tile([NBLK * CPC, SFREE], mybir.dt.float32,
                             name=f"stg{i}", tag=f"stg{i}")[:, :] for i in range(NB)]

            def emit_load_group(r0, n):
                for blk in range(NBLK):
                    src = AP(xb.tensor, xb.offset + (blk * BSTEP + r0) * PH,
                             [[PIMG, CPC], [1, n * PH]])
                    dst = AP(timg.tensor,
                             timg.offset + blk * CPC * FIMG + r0 * PH,
                             [[FIMG, CPC], [1, n * PH]])
                    nc.gpsimd.dma_start(out=dst, in_=src)

            def emit_copy(w, ch):
                kh, kw = divmod(w, 3)
                s = stg[w % NB]
                csrc = AP(timg.tensor,
                          timg.offset + (kh + ch * CROW) * PH + kw,
                          [[FIMG, NBLK * CPC], [PH, CROW], [1, OH]])
                cdst = AP(s.tensor, s.offset + ch * CHST,
                          [[SFREE, NBLK * CPC], [OH, CROW], [1, OH]])
                if (w + ch) % 2 == 0:
                    nc.scalar.copy(out=cdst, in_=csrc)
                else:
                    nc.vector.tensor_copy(cdst, csrc)

            def emit_store(w, blk, ch0, nrows):
                kh, kw = divmod(w, 3)
                s = stg[w % NB]
                nfull, rem = divmod(nrows, CROW)
                base_s = s.offset + blk * CPC * SFREE + ch0 * CHST
                base_d = (ob.offset + (kh * 3 + kw) * OSZ
                          + (blk * BSTEP + ch0 * CROW) * OH)
                if nfull:
                    ssrc = AP(s.tensor, base_s,
                              [[SFREE, CPC], [CHST, nfull], [1, CROW * OH]])
                    sdst = AP(ob.tensor, base_d,
                              [[9 * OSZ, CPC], [CROW * OH, nfull], [1, CROW * OH]])
                    nc.gpsimd.dma_start(out=sdst, in_=ssrc)
                if rem:
                    ssrc = AP(s.tensor, base_s + nfull * CHST,
                              [[SFREE, CPC], [1, rem * OH]])
                    sdst = AP(ob.tensor, base_d + nfull * CROW * OH,
                              [[9 * OSZ, CPC], [1, rem * OH]])
                    nc.gpsimd.dma_start(out=sdst, in_=ssrc)

            # group0: rows 0-28 of each block. Lands alone (~17 us) so the
            # kh=0 windows' low chunks can copy + store while group1 flies.
            emit_load_group(0, CROW + 1)
            for w in range(3):
                emit_copy(w, 0)                       # phase A copies (kh=0)
            emit_store(0, 0, 0, CROW)
            emit_store(0, 1, 0, CROW)
            # group1: rows 28-57; rewrites row 28 (same DRAM data) so the
            # WAW overlap orders it after group0 instead of interleaving.
            emit_load_group(CROW, BR - CROW)
            emit_store(0, 2, 0, CROW)
            emit_store(0, 3, 0, CROW)
            for w in range(1, 3):
                for blk in range(NBLK):
                    emit_store(w, blk, 0, CROW)
            # phase B: high chunks of the kh=0 windows.
            for w in range(3):
                emit_copy(w, 1)
            for w in range(3):
                for blk in range(NBLK):
                    nv = min(BSTEP, OH - blk * BSTEP)
                    emit_store(w, blk, 1, nv - CROW)
            # windows 3-8: straightforward copy -> store per window.
            for w in range(3, 9):
                emit_copy(w, 0)
                emit_copy(w, 1)
                for blk in range(NBLK):
                    nv = min(BSTEP, OH - blk * BSTEP)
                    emit_store(w, blk, 0, nv)
    return nc


def _split_multi_waits(nc: bass.Bass) -> None:
    """Walrus allows only one sync-wait command per instruction (the
    kernel-tail drain ends up with one per DMA-completion sem lane).
    Hoist all but the last wait onto fresh single-wait NOPs inserted
    just before the instruction on the same engine — semantically
    identical (the engine blocks on each wait in turn)."""
    from bass_rust import SyncInfo

    k = 0
    for fn in nc.m.functions:
        for blk in fn.blocks:
            insts = blk.instructions
            for idx in range(len(insts) - 1, -1, -1):
                inst = insts[idx]
                si = inst.sync_info
                if si is None or len(si.on_wait) <= 1:
                    continue
                waits = list(si.on_wait)
                for w in waits[:-1]:
                    nop = mybir.InstNoOp(name=f"WSPLIT-{k}")
                    k += 1
                    nop.engine = inst.engine
                    nop.sync_info = SyncInfo(on_wait=[w], on_update=[])
                    insts.insert(idx, nop)
                si.on_wait = [waits[-1]]
                inst.sync_info = si


def get_nc() -> bass.Bass:
    if "nc" not in _NC_CACHE:
        nc = build_nc()
        _split_multi_waits(nc)
        _NC_CACHE["nc"] = nc
    return _NC_CACHE["nc"]


def make_in_maps(x: np.ndarray) -> list[dict]:
    x = np.asarray(x, dtype=np.float32)
    xp = np.pad(x, ((0, 0), (0, 0), (1, 1), (1, 1)))
    maps = []
    for core in range(N_CORES):
        b, half = divmod(core, 2)
        maps.append({"xp": np.ascontiguousarray(xp[b, half * CPC:(half + 1) * CPC])})
    return maps


def gather_out(results: list[dict]) -> np.ndarray:
    out = np.empty((B, C * 9, OSZ), dtype=np.float32)
    for core in range(N_CORES):
        b, half = divmod(core, 2)
        out[b, half * NROW:(half + 1) * NROW] = results[core]["out"]
    return out


def kernel(**inputs) -> np.ndarray:
    x = inputs["x"]
    nc = get_nc()
    res = run_bass_kernel_spmd(nc, make_in_maps(x), list(range(N_CORES)))
    return gather_out(res.results)


# revision 13
# speedup vs baseline: 1.2875x; 1.1034x over previous
"""Unfold/im2col kernel for Trainium2 (Bass/Tile), 8-core data parallel.

Problem: x [4, 64, 224, 224] f32 -> out [4, 576, 49729] f32 where
out[b, (c*3+kh)*3+kw, oh*223+ow] = pad(x,1)[b, c, oh+kh, ow+kw]
(3x3 kernel, pad 1, stride 1, dilation 1, oh=ow=223).

Sharding: 8 cores = (batch 4) x (channel half 2). Each core handles
32 channels -> [288, 49729] independently; outputs concatenate on the
channel axis (channel-major row layout makes halves contiguous).

Layout strategy: the old kernel stored straight from a row-partitioned
image, capping every store descriptor at one 223-element window row
(892 B); SDMA per-descriptor overhead then limited aggregate BW
(308 us measured). Here the padded input is loaded with partition =
(row-block, channel): each of the 128 partitions holds 58 padded rows
x 226 of one channel (4 blocks of 56 output rows cover oh=223 with a
2-row halo). The scalar (Activation) and vector (DVE) engines then
materialize each of the 9 (kh, kw) shifted windows into staging tiles
where output rows are contiguous per partition; stores move CROW=28
output rows (25 KB) per descriptor. ~50 KB descriptors crash the
device (NRT level, found empirically); 12.5-27 KB are safe.

Descriptor sizing: loads are split into separate chunk DMAs; staging
chunks are separated by a 1-element gap so the DMA AP optimizer cannot
re-merge them into bigger descriptors (the symbolic-AP lowering path
ignores max_dma_last_dim).

Schedule (from trace analysis: SDMA engines are ~99% busy start to
finish, and per-descriptor throughput ramps 13 -> 22.5 B/ns/engine
over ~200 us regardless of workload content — a device warm-up we can
only ride, not schedule away; stores previously could not start until
~40 us because all 8 load DMAs' descriptors interleave across the 16
engines and so all complete together):
  - load group0 (padded rows 0-28) first; group1 (rows 28-57)
    deliberately rewrites row 28 so the WAW overlap forces Tile to
    serialize it after group0 -> group0 lands ~17 us.
  - staging is 6 chunk-granular slots on a conveyor: the six kh<=1
    low chunks copy first (they only need group0), so stores flow
    continuously from ~20 us with no phase barriers; group1's
    transfers overlap the early store stream.
  - copies alternate between the Activation and DVE engines; slot
    reuse (WAR on the slot) paces copies ~6 stores ahead.
"""

from contextlib import ExitStack

import numpy as np

import concourse.bass as bass
import concourse.tile as tile
from concourse import mybir
from concourse.ap import AP
from concourse.bass_utils import run_bass_kernel_spmd

B, C, IH, IW = 4, 64, 224, 224
N_CORES = 8
CPC = C // 2          # channels per core: 32
PH = IH + 2           # padded height/width: 226
OH = IH - 1           # output spatial: 223
OSZ = OH * OH         # 49729
NROW = CPC * 9        # 288 output rows per core
PIMG = PH * PH        # padded image elements: 51076
NBLK = 4              # output-row blocks (partition = blk*32 + c)
BSTEP = 56            # output rows per block (last block: 55)
BR = BSTEP + 2        # padded rows loaded per partition: 58
FIMG = BR * PH        # input free elems per partition: 13108

CROW = 28             # staged rows per store chunk (28*223*4 = 24976 B)
NSLOT = 6             # chunk-granular staging slots
CHST = CROW * OH + 1  # slot elems per partition (gap elem stops dim-merge)

_NC_CACHE = {}


def build_nc() -> bass.Bass:
    nc = bass.Bass(dynamic_dma_scratch_size=4096)
    x = nc.declare_dram_parameter("xp", [CPC, PH, PH], mybir.dt.float32, isOutput=False)
    out = nc.declare_dram_parameter("out", [NROW, OSZ], mybir.dt.float32, isOutput=True)
    xb = x[:, :, :]
    ob = out[:, :]

    with tile.TileContext(nc) as tc:
        with ExitStack() as ctx:
            pool = ctx.enter_context(tc.tile_pool(name="p", bufs=1))
            timg = pool.tile([NBLK * CPC, FIMG], mybir.dt.float32,
                             name="timg", tag="timg")[:, :]
            slots = [pool.tile([NBLK * CPC, CHST], mybir.dt.float32,
                               name=f"sl{i}", tag=f"sl{i}")[:, :]
                     for i in range(NSLOT)]

            def emit_load_group(r0, n):
                for blk in range(NBLK):
                    src = AP(xb.tensor, xb.offset + (blk * BSTEP + r0) * PH,
                             [[PIMG, CPC], [1, n * PH]])
                    dst = AP(timg.tensor,
                             timg.offset + blk * CPC * FIMG + r0 * PH,
                             [[FIMG, CPC], [1, n * PH]])
                    nc.gpsimd.dma_start(out=dst, in_=src)

            def emit_copy(s, w, ch, eng):
                kh, kw = divmod(w, 3)
                csrc = AP(timg.tensor,
                          timg.offset + (kh + ch * CROW) * PH + kw,
                          [[FIMG, NBLK * CPC], [PH, CROW], [1, OH]])
                cdst = AP(s.tensor, s.offset,
                          [[CHST, NBLK * CPC], [OH, CROW], [1, OH]])
                if eng == 0:
                    nc.scalar.copy(out=cdst, in_=csrc)
                else:
                    nc.vector.tensor_copy(cdst, csrc)

            def emit_stores(s, w, ch):
                kh, kw = divmod(w, 3)
                for blk in range(NBLK):
                    nv = min(BSTEP, OH - blk * BSTEP)       # 56,56,56,55
                    nrows = min(CROW, nv - ch * CROW)       # 28 (27: blk3 ch1)
                    base_d = (ob.offset + (kh * 3 + kw) * OSZ
                              + (blk * BSTEP + ch * CROW) * OH)
                    ssrc = AP(s.tensor, s.offset + blk * CPC * CHST,
                              [[CHST, CPC], [1, nrows * OH]])
                    sdst = AP(ob.tensor, base_d,
                              [[9 * OSZ, CPC], [1, nrows * OH]])
                    nc.gpsimd.dma_start(out=sdst, in_=ssrc)

            # Conveyor: 18 (window, chunk) copies in dependency order — the
            # six kh<=1 low chunks first (they only need load group0), high
            # chunks and kh=2 interleaved after.  Each copy goes to the next
            # slot round-robin (WAR on the slot paces copies behind stores)
            # and alternates engines; its stores follow immediately.
            seq = [(0, 0), (1, 0), (2, 0), (3, 0), (4, 0), (5, 0),
                   (0, 1), (1, 1), (2, 1), (6, 0), (3, 1), (7, 0),
                   (4, 1), (8, 0), (5, 1), (6, 1), (7, 1), (8, 1)]
            emit_load_group(0, CROW + 1)          # rows 0-28, lands ~17 us
            for p, (w, ch) in enumerate(seq):
                s = slots[p % NSLOT]
                emit_copy(s, w, ch, p % 2)
                emit_stores(s, w, ch)
                if p == 0:
                    # group1: rows 28-57; rewrites row 28 (same DRAM data) so
                    # the WAW overlap orders it after group0 instead of both
                    # load groups' descriptors interleaving across engines.
                    emit_load_group(CROW, BR - CROW)
    return nc


def _split_multi_waits(nc: bass.Bass) -> None:
    """Walrus allows only one sync-wait command per instruction (the
    kernel-tail drain ends up with one per DMA-completion sem lane).
    Hoist all but the last wait onto fresh single-wait NOPs inserted
    just before the instruction on the same engine — semantically
    identical (the engine blocks on each wait in turn)."""
    from bass_rust import SyncInfo

    k = 0
    for fn in nc.m.functions:
        for blk in fn.blocks:
            insts = blk.instructions
            for idx in range(len(insts) - 1, -1, -1):
                inst = insts[idx]
                si = inst.sync_info
                if si is None or len(si.on_wait) <= 1:
                    continue
                waits = list(si.on_wait)
                for w in waits[:-1]:
                    nop = mybir.InstNoOp(name=f"WSPLIT-{k}")
                    k += 1
                    nop.engine = inst.engine
                    nop.sync_info = SyncInfo(on_wait=[w], on_update=[])
                    insts.insert(idx, nop)
                si.on_wait = [waits[-1]]
                inst.sync_info = si


def get_nc() -> bass.Bass:
    if "nc" not in _NC_CACHE:
        nc = build_nc()
        _split_multi_waits(nc)
        _NC_CACHE["nc"] = nc
    return _NC_CACHE["nc"]


def make_in_maps(x: np.ndarray) -> list[dict]:
    x = np.asarray(x, dtype=np.float32)
    xp = np.pad(x, ((0, 0), (0, 0), (1, 1), (1, 1)))
    maps = []
    for core in range(N_CORES):
        b, half = divmod(core, 2)
        maps.append({"xp": np.ascontiguousarray(xp[b, half * CPC:(half + 1) * CPC])})
    return maps


def gather_out(results: list[dict]) -> np.ndarray:
    out = np.empty((B, C * 9, OSZ), dtype=np.float32)
    for core in range(N_CORES):
        b, half = divmod(core, 2)
        out[b, half * NROW:(half + 1) * NROW] = results[core]["out"]
    return out


def kernel(**inputs) -> np.ndarray:
    x = inputs["x"]
    nc = get_nc()
    res = run_bass_kernel_spmd(nc, make_in_maps(x), list(range(N_CORES)))
    return gather_out(res.results)
